# revision 5
# baseline (speedup 1.0000x reference)
"""Trainium2 Bass kernel for nn_CompositionalMlp (4-node compositional MLP,
4 experts/node, exact one-hot routing), data-parallel over batch on 8 cores.

Routing-sorted design: the host reads the one-hot routing blocks, pairs rows
with identical (e0,e1,e2,e3) routes, and packs pairs into chunks of 512 rows
such that at EVERY node each chunk splits into 4 expert segments of exactly
128 rows. Each layer then runs only the selected expert's weights per segment
(quarter the dense MAC count, no masks / predicated selection). Between nodes
the running activation x_prev is re-permuted on-device with ap_gather on the
otherwise-idle GPSIMD engine (pairs move as fp32 units via bitcast). Relus are
split column-wise across DVE and the scalar engine so PE stays the bottleneck.
"""
import os
import sys
sys.path.insert(0, "/opt/trn_rl_repo")
os.environ.setdefault("NEURON_RT_RESET_CORES", "1")
import numpy as np

B = 65536
E = 4
F = 32
H = 256
NODES = 4
D_MID = 128
D_OUT = 8
N_CORES = 8
BC = B // N_CORES      # 8192 rows per core
CH = 1024              # batch columns (row-slots) per chunk
PCH = CH // 2          # pair-slots per chunk
SEG = 256              # row-slots per expert segment
PSEG = SEG // 2        # pair-slots per expert segment
IDXW = 3 * (PCH // 16)  # idx cols per chunk (3 transitions)

# packed weight-matrix column offsets (fp16, [128, WCOLS]).
# input/pre weights sit at partitions 32j:32j+32 (same base partition as the
# moving operand xt[32j:...] -- a matmul hardware requirement).
WIN_OFF = 0            # node j at 1024j + 256e + 128h, parts 32j..  -> 4096 cols
WA_OFF = 4096          # 12 x [128, 256]                     -> 3072
WB_OFF = 7168          # 12 x 2 ktiles x [128, 256]          -> 6144
WU_OFF = 13312         # 12 x 2 ktiles x [128, 128]          -> 3072
W3_OFF = 16384         # 4 experts x 2 ktiles x [128, 8]     -> 64
WCOLS = 16448

# bias-vector columns (fp32, [128, 17])
def _BIN(j, h): return 2 * j + h          # input/pre layer, per (node, half)
def _BIF(j, h): return 8 + 2 * (j - 1) + h  # interface layer, nodes 1-3
def _BCB(j): return 14 + j                 # combine layer, nodes 0-2
NBV = 17

_COMPILED = {}
_LAST_META = {}


def _build(n_chunks: int):
    import concourse.bass as bass  # noqa: F401
    from concourse import bacc
    import concourse.mybir as mybir
    from concourse.tile import TileContext

    F16 = mybir.dt.float16
    F32 = mybir.dt.float32
    I16 = mybir.dt.int16
    ADD = mybir.AluOpType.add
    MAX = mybir.AluOpType.max
    RELU = mybir.ActivationFunctionType.Relu
    IDENT = mybir.ActivationFunctionType.Identity

    S = n_chunks * CH
    nc = bacc.Bacc("TRN2", target_bir_lowering=False, debug=False,
                   num_devices=N_CORES)
    # [64, 1024] per chunk: nodes 0/1 at partitions 0/32 in cols 0:512,
    # nodes 2/3 in cols 512:1024 (matmul base partition must be 0/32/64)
    xin = nc.dram_tensor("xin", [n_chunks, 64, 2 * CH], F16, kind="ExternalInput").ap()
    idx = nc.dram_tensor("idx", [128, IDXW * n_chunks], I16, kind="ExternalInput").ap()
    wm = nc.dram_tensor("wm", [128, WCOLS], F16, kind="ExternalInput").ap()
    bv = nc.dram_tensor("bv", [128, NBV], F32, kind="ExternalInput").ap()
    b3 = nc.dram_tensor("b3", [8, 4], F32, kind="ExternalInput").ap()
    yT = nc.dram_tensor("yT", [8, S], F32, kind="ExternalOutput").ap()

    with TileContext(nc) as tc:
        with (
            tc.tile_pool(name="wpool", bufs=1) as wp,
            tc.tile_pool(name="io", bufs=3) as io,
            tc.tile_pool(name="psab", bufs=3, space="PSUM") as psab,
            tc.tile_pool(name="psu", bufs=2, space="PSUM") as psu,
        ):
            # DMA order: first-node weights + biases + inputs land before the
            # bulk weight transfers so the first matmuls start early
            wmt = wp.tile([128, WCOLS], F16, tag="wm")
            bvt = wp.tile([128, NBV], F32, tag="bv")
            b3t = wp.tile([8, 4], F32, tag="b3")
            XT = wp.tile([64, 2 * CH * n_chunks], F16, tag="XT")
            IT = wp.tile([128, IDXW * n_chunks], I16, tag="IT")
            nc.sync.dma_start(wmt[:, 0:1024], wm[:, 0:1024])
            nc.sync.dma_start(bvt[:, :], bv[:, :])
            for ci in range(n_chunks):
                nc.sync.dma_start(XT[:, 2 * CH * ci:2 * CH * (ci + 1)], xin[ci])
            nc.sync.dma_start(wmt[:, 13312:16384], wm[:, 13312:16384])
            nc.sync.dma_start(wmt[:, 1024:4096], wm[:, 1024:4096])
            nc.sync.dma_start(wmt[:, 4096:7168], wm[:, 4096:7168])
            nc.sync.dma_start(wmt[:, 7168:13312], wm[:, 7168:13312])
            nc.sync.dma_start(wmt[:, 16384:WCOLS], wm[:, 16384:WCOLS])
            nc.sync.dma_start(IT[:, :], idx[:, :])
            nc.sync.dma_start(b3t[:, :], b3[:, :])
            HP = wp.tile([128, 2 * CH * n_chunks], F16, tag="HP")
            HM = wp.tile([128, 2 * CH * n_chunks], F16, tag="HM")
            XQ = wp.tile([128, S], F16, tag="XQ")
            XP = wp.tile([128, S], F16, tag="XP")

            def WINap(j, e, h):
                c = WIN_OFF + 1024 * j + 256 * e + 128 * h
                pb = 32 * (j % 2)
                return wmt[pb:pb + 32, c:c + 128]

            def WAap(i12, h):
                c = WA_OFF + 256 * i12 + 128 * h
                return wmt[:, c:c + 128]

            def WBap(i12, k, h):
                c = WB_OFF + 512 * i12 + 256 * k + 128 * h
                return wmt[:, c:c + 128]

            def WUap(i12, k):
                c = WU_OFF + 256 * i12 + 128 * k
                return wmt[:, c:c + 128]

            def W3ap(e, k):
                c = W3_OFF + 16 * e + 8 * k
                return wmt[:, c:c + 8]

            def relu_split(dst, src, ncols, bcol_a, bcol_b):
                """dst/src [128, ncols]; halves at ncols//2 boundary get their
                own bias column; DVE takes half A, scalar engine half B."""
                hn = ncols // 2
                nc.vector.tensor_scalar(dst[:, 0:hn], src[:, 0:hn],
                                        bvt[:, bcol_a:bcol_a + 1], 0.0, ADD, MAX)
                nc.scalar.activation(dst[:, hn:ncols], src[:, hn:ncols], RELU,
                                     bias=bvt[:, bcol_b:bcol_b + 1])

            def win_c(ci):
                return slice(2 * CH * ci, 2 * CH * (ci + 1))

            def phase_pre(j, cis=None):
                """input / pre layer: XT -> HP."""
                pb = 32 * (j % 2)
                for ci in (range(n_chunks) if cis is None else cis):
                    co = 2 * CH * ci + CH * (j // 2)
                    hb = 2 * CH * ci
                    for w in range(2):
                        pp = psab.tile([128, 1024], F32, tag="pa")
                        for s2 in range(2):
                            e = 2 * w + s2
                            mv = XT[pb:pb + 32, co + SEG * e:co + SEG * (e + 1)]
                            for h in range(2):
                                nc.tensor.matmul(
                                    pp[:, 512 * h + 256 * s2:512 * h + 256 * (s2 + 1)],
                                    WINap(j, e, h), mv, start=True, stop=True)
                        nc.vector.tensor_scalar(
                            HP[:, hb + 512 * w:hb + 512 * (w + 1)], pp[:, 0:512],
                            bvt[:, _BIN(j, 0):_BIN(j, 0) + 1], 0.0, ADD, MAX)
                        nc.scalar.activation(
                            HP[:, hb + CH + 512 * w:hb + CH + 512 * (w + 1)],
                            pp[:, 512:1024], RELU,
                            bias=bvt[:, _BIN(j, 1):_BIN(j, 1) + 1])

            def phase_iface(j):
                """interface layer for all chunks: (XP | HP) -> HM, K=384."""
                for ci in range(n_chunks):
                    hb = 2 * CH * ci
                    for w in range(2):
                        pi = psab.tile([128, 1024], F32, tag="pa")
                        for s2 in range(2):
                            e = 2 * w + s2
                            i12 = 4 * (j - 1) + e
                            sg = slice(hb + SEG * e, hb + SEG * (e + 1))
                            sgB = slice(hb + CH + SEG * e, hb + CH + SEG * (e + 1))
                            for h in range(2):
                                ww = pi[:, 512 * h + 256 * s2:512 * h + 256 * (s2 + 1)]
                                nc.tensor.matmul(
                                    ww, WAap(i12, h),
                                    XP[:, CH * ci + SEG * e:CH * ci + SEG * (e + 1)],
                                    start=True, stop=False)
                                nc.tensor.matmul(ww, WBap(i12, 0, h), HP[:, sg],
                                                 start=False, stop=False)
                                nc.tensor.matmul(ww, WBap(i12, 1, h), HP[:, sgB],
                                                 start=False, stop=True)
                        nc.vector.tensor_scalar(
                            HM[:, hb + 512 * w:hb + 512 * (w + 1)], pi[:, 0:512],
                            bvt[:, _BIF(j, 0):_BIF(j, 0) + 1], 0.0, ADD, MAX)
                        nc.scalar.activation(
                            HM[:, hb + CH + 512 * w:hb + CH + 512 * (w + 1)],
                            pi[:, 512:1024], RELU,
                            bias=bvt[:, _BIF(j, 1):_BIF(j, 1) + 1])

            def phase_comb(j, src_t, cis=None):
                """combine H->D_MID + relu + permute-to-next-order, per chunk."""
                IW = PCH // 16
                for ci in (range(n_chunks) if cis is None else cis):
                    hb = 2 * CH * ci
                    for w in range(2):
                        pu = psu.tile([128, 512], F32, tag="pu")
                        for s2 in range(2):
                            e = 2 * w + s2
                            sl = slice(256 * s2, 256 * (s2 + 1))
                            nc.tensor.matmul(pu[:, sl], WUap(4 * j + e, 0),
                                             src_t[:, hb + SEG * e:hb + SEG * (e + 1)],
                                             start=True, stop=False)
                            nc.tensor.matmul(pu[:, sl], WUap(4 * j + e, 1),
                                             src_t[:, hb + CH + SEG * e:hb + CH + SEG * (e + 1)],
                                             start=False, stop=True)
                        xq = XQ[:, CH * ci + 512 * w:CH * ci + 512 * (w + 1)]
                        nc.vector.tensor_scalar(xq[:, 0:256], pu[:, 0:256],
                                                bvt[:, _BCB(j):_BCB(j) + 1],
                                                0.0, ADD, MAX)
                        nc.scalar.activation(xq[:, 256:512], pu[:, 256:512], RELU,
                                             bias=bvt[:, _BCB(j):_BCB(j) + 1])
                    nc.gpsimd.ap_gather(
                        XP[:, CH * ci:CH * (ci + 1)].bitcast(F32),
                        XQ[:, CH * ci:CH * (ci + 1)].bitcast(F32),
                        IT[:, IDXW * ci + IW * j:IDXW * ci + IW * (j + 1)],
                        channels=128, num_elems=PCH, d=1, num_idxs=PCH)

            def phase_head():
                for ci in range(n_chunks):
                    hb = 2 * CH * ci
                    for w in range(2):
                        put = psu.tile([128, 512], F32, tag="pu")
                        ph = put[0:8, :]
                        for s2 in range(2):
                            e = 2 * w + s2
                            sl = slice(256 * s2, 256 * (s2 + 1))
                            nc.tensor.matmul(ph[:, sl], W3ap(e, 0),
                                             HM[:, hb + SEG * e:hb + SEG * (e + 1)],
                                             start=True, stop=False)
                            nc.tensor.matmul(ph[:, sl], W3ap(e, 1),
                                             HM[:, hb + CH + SEG * e:hb + CH + SEG * (e + 1)],
                                             start=False, stop=True)
                        # head bias applied host-side; one copy psum->sbuf
                        yt = io.tile([8, 512], F32, tag="yt")
                        nc.scalar.copy(yt[:, :], ph[:, :])
                        nc.sync.dma_start(
                            yT[:, CH * ci + 512 * w:CH * ci + 512 * (w + 1)],
                            yt[:, :])

            # interleave comb(j) with pre(j+1) per chunk: independent work
            # keeps the PE FIFO fed across comb's psum-ring/relu waits
            phase_pre(0)
            for j in (0, 1, 2):
                for ci in range(n_chunks):
                    phase_comb(j, HP if j == 0 else HM, [ci])
                    phase_pre(j + 1, [ci])
                phase_iface(j + 1)
            phase_head()
    nc.compile()
    return nc


def _route_pack(eids, n_chunks):
    """eids [4, R] per-node expert ids. Returns (pairs_chunk, slots[4],
    pairs_r0, pairs_r1) with exactly PSEG pairs per (chunk, node, expert)
    cell (pads included), or None if the greedy packing fails."""
    R = eids.shape[1]
    t4 = ((eids[0] * 4 + eids[1]) * 4 + eids[2]) * 4 + eids[3]
    NC = n_chunks
    used = np.zeros((NC, 4, 4), np.int32)
    tot = np.zeros(NC, np.int32)

    tcnt = np.bincount(t4, minlength=256)
    # decreasing pair-count order helps the greedy packing
    torder = np.argsort(-tcnt, kind="stable")
    alloc = np.zeros((256, NC), np.int32)
    for tt in torder:
        npair = (int(tcnt[tt]) + 1) // 2
        if npair == 0:
            continue
        d = [(tt >> 6) & 3, (tt >> 4) & 3, (tt >> 2) & 3, tt & 3]
        slack = np.minimum.reduce([PSEG - used[:, j, d[j]] for j in range(4)])
        slack = np.minimum(slack, PCH - tot)
        if int(slack.sum()) < npair:
            return None
        a = np.minimum(slack, npair // NC)
        rem = npair - int(a.sum())
        while rem > 0:
            room = slack - a
            cand = np.flatnonzero(room > 0)
            take = cand[np.argsort(-room[cand], kind="stable")[:rem]]
            a[take] += 1
            rem -= len(take)
        for j in range(4):
            used[:, j, d[j]] += a
        tot += a
        alloc[tt] = a

    # per-pair arrays: reals first (grouped by type, then chunk), pads after
    rows_by_type = np.argsort(t4, kind="stable")
    tstart = np.zeros(257, np.int64)
    np.cumsum(tcnt, out=tstart[1:])
    pr0, pr1, pch, pd = [], [], [], []
    for tt in range(256):
        npair = (int(tcnt[tt]) + 1) // 2
        if npair == 0:
            continue
        rows = rows_by_type[tstart[tt]:tstart[tt + 1]]
        r0 = rows[0::2]
        r1 = np.full(npair, -1, np.int64)
        r1[:len(rows) - len(r0)] = rows[1::2]
        pr0.append(r0)
        pr1.append(r1)
        pch.append(np.repeat(np.arange(NC), alloc[tt]))
        pd.append(np.tile([(tt >> 6) & 3, (tt >> 4) & 3, (tt >> 2) & 3, tt & 3],
                          (npair, 1)))
    # pad pairs fill every remaining cell deficit
    for c in range(NC):
        npad = PCH - int(tot[c])
        if npad == 0:
            continue
        dcols = []
        for j in range(4):
            deficit = PSEG - used[c, j]
            dcols.append(np.repeat(np.arange(4), deficit))
        pr0.append(np.full(npad, -1, np.int64))
        pr1.append(np.full(npad, -1, np.int64))
        pch.append(np.full(npad, c, np.int64))
        pd.append(np.stack(dcols, axis=1))
    pairs_r0 = np.concatenate(pr0)
    pairs_r1 = np.concatenate(pr1)
    pairs_chunk = np.concatenate(pch).astype(np.int64)
    pairs_d = np.concatenate(pd, axis=0)
    P = len(pairs_r0)
    assert P == NC * PCH

    slots = []
    for j in range(4):
        order = np.lexsort((np.arange(P), pairs_d[:, j], pairs_chunk))
        key = pairs_chunk[order] * 4 + pairs_d[order, j]
        # rank within each (chunk, expert) group
        grp_change = np.empty(P, bool)
        grp_change[0] = True
        grp_change[1:] = key[1:] != key[:-1]
        gidx = np.cumsum(grp_change) - 1
        starts = np.flatnonzero(grp_change)
        rank = np.arange(P) - starts[gidx]
        sj = np.empty(P, np.int64)
        sj[order] = PSEG * pairs_d[order, j] + rank
        assert rank.max() < PSEG
        slots.append(sj)
    return pairs_chunk, slots, pairs_r0, pairs_r1


def _wrap_idx(idxp):
    """[NC, PCH] -> wrapped int16 [NC, 128, PCH//16] per ap_gather's layout."""
    NC = idxp.shape[0]
    w = idxp.reshape(NC, PCH // 16, 16).transpose(0, 2, 1)   # [NC, 16, s]
    return np.tile(w, (1, 8, 1)).astype(np.int16)


def _prep_inputs(p):
    f32 = np.float32
    f16 = np.float16
    x = np.asarray(p["input_val"], f32)

    # ---- packed weights [128, WCOLS] fp16 ----
    wmat = np.zeros((128, WCOLS), f32)
    in_w = [p["W0_0"], p["W1_pre"], p["W2_pre"], p["W3_pre"]]
    in_b = [p["b0_0"], p["b1_pre"], p["b2_pre"], p["b3_pre"]]
    bvec = np.zeros((128, NBV), f32)
    for j in range(4):
        for e in range(4):
            c = WIN_OFF + 1024 * j + 256 * e
            pb = 32 * (j % 2)
            wmat[pb:pb + 32, c:c + 256] = in_w[j][e]
        bj = np.asarray(in_b[j], f32)
        assert np.ptp(bj, axis=0).max() == 0.0
        for h in range(2):
            bvec[:, _BIN(j, h)] = bj[0][128 * h:128 * (h + 1)]
    for j in (1, 2, 3):
        w0 = np.asarray(p[f"W{j}_0"], f32)
        b0 = np.asarray(p[f"b{j}_0"], f32)
        assert np.ptp(b0, axis=0).max() == 0.0
        for e in range(4):
            i12 = 4 * (j - 1) + e
            wmat[:, WA_OFF + 256 * i12:WA_OFF + 256 * (i12 + 1)] = w0[e][0:128]
            for k in range(2):
                wmat[:, WB_OFF + 512 * i12 + 256 * k:WB_OFF + 512 * i12 + 256 * (k + 1)] = \
                    w0[e][128 + 128 * k:128 + 128 * (k + 1)]
        for h in range(2):
            bvec[:, _BIF(j, h)] = b0[0][128 * h:128 * (h + 1)]
    for j in (0, 1, 2):
        w1 = np.asarray(p[f"W{j}_1"], f32)
        b1 = np.asarray(p[f"b{j}_1"], f32)
        assert np.ptp(b1, axis=0).max() == 0.0
        for e in range(4):
            i12 = 4 * j + e
            for k in range(2):
                wmat[:, WU_OFF + 256 * i12 + 128 * k:WU_OFF + 256 * i12 + 128 * (k + 1)] = \
                    w1[e][128 * k:128 * (k + 1)]
        bvec[:, _BCB(j)] = b1[0]
    w3 = np.asarray(p["W3_1"], f32)
    for e in range(4):
        for k in range(2):
            wmat[0:128, W3_OFF + 16 * e + 8 * k:W3_OFF + 16 * e + 8 * (k + 1)] = \
                w3[e][128 * k:128 * (k + 1)]
    b3t = np.zeros((8, 4), f32)
    b3t[:, :] = np.asarray(p["b3_1"], f32).T

    # ---- routing: expert id per node per row ----
    ohs = x[:, NODES * F:].reshape(B, NODES, E)
    eids_all = np.argmax(ohs, axis=2).astype(np.int32)    # [B, 4]
    feats = x[:, :NODES * F]                              # [B, 128]

    # all cores must share one program => one n_chunks; bump globally on failure
    nch = 9
    packs = None
    while packs is None:
        packs = []
        for c in range(N_CORES):
            pk = _route_pack(eids_all[c * BC:(c + 1) * BC].T, nch)
            if pk is None:
                packs = None
                nch += 1
                assert nch <= 24, "route packing failed"
                break
            packs.append(pk)

    in_maps = []
    outcols = []
    shared = dict(wm=wmat.astype(f16), bv=bvec, b3=b3t)
    for c in range(N_CORES):
        rs = slice(c * BC, (c + 1) * BC)
        pairs_chunk, slots, pr0, pr1 = packs[c]
        S = nch * CH
        fcore = feats[rs]                                 # [BC, 128]

        xinb = np.zeros((64, nch * 2 * CH), f32)
        m0 = pr0 >= 0
        m1 = pr1 >= 0
        for j in range(4):
            pb = 32 * (j % 2)
            col = 2 * CH * pairs_chunk + CH * (j // 2) + 2 * slots[j]
            xinb[pb:pb + 32, col[m0]] = fcore[pr0[m0], 32 * j:32 * (j + 1)].T
            xinb[pb:pb + 32, col[m1] + 1] = fcore[pr1[m1], 32 * j:32 * (j + 1)].T
        xinb = xinb.reshape(64, nch, 2 * CH).transpose(1, 0, 2)

        IW = PCH // 16
        idxb = np.zeros((nch, 128, IDXW), np.int16)
        for t in range(3):
            arr = np.zeros((nch, PCH), np.int64)
            arr[pairs_chunk, slots[t + 1]] = slots[t]
            idxb[:, :, IW * t:IW * (t + 1)] = _wrap_idx(arr)
        idxb = np.ascontiguousarray(
            idxb.transpose(1, 0, 2).reshape(128, IDXW * nch))

        oc = np.zeros(BC, np.int64)
        col3 = CH * pairs_chunk + 2 * slots[3]
        m0 = pr0 >= 0
        oc[pr0[m0]] = col3[m0]
        m1 = pr1 >= 0
        oc[pr1[m1]] = col3[m1] + 1
        outcols.append(oc)

        m = dict(shared)
        m["xin"] = np.ascontiguousarray(xinb.astype(f16))
        m["idx"] = idxb
        in_maps.append(m)

    _LAST_META["outcols"] = outcols
    _LAST_META["n_chunks"] = nch
    _LAST_META["e3"] = eids_all[:, 3].astype(np.int64)
    _LAST_META["b3"] = np.asarray(p["b3_1"], np.float32)
    return in_maps


def kernel(**inputs):
    from concourse.bass_utils import run_bass_kernel_spmd

    in_maps = _prep_inputs({k: np.asarray(v) for k, v in inputs.items()})
    nch = _LAST_META["n_chunks"]
    key = ("nc", nch)
    if key not in _COMPILED:
        _COMPILED[key] = _build(nch)
        _COMPILED[("nc", True)] = _COMPILED[key]   # test.py compat alias
    nc = _COMPILED[key]
    res = run_bass_kernel_spmd(nc, in_maps, core_ids=list(range(N_CORES)))
    out = np.empty((B, D_OUT), np.float32)
    for c in range(N_CORES):
        yc = res.results[c]["yT"]                  # [8, S]
        out[c * BC:(c + 1) * BC] = yc[:, _LAST_META["outcols"][c]].T
    out += _LAST_META["b3"][_LAST_META["e3"]]      # head bias, host-side
    return out
